# revision 29
# baseline (speedup 1.0000x reference)
"""AttnBlock2D Trainium2 kernel.

Reference computation (per batch element b):
    q = Wq @ x_self + bq            (1x1 conv == per-pixel linear)
    k = Wk @ x_cross + bk
    v = Wv @ x_cross + bv
    per head h (8 heads, head_dim 64, n = 32*32 = 1024 pixels):
        scores = q_h^T k_h / 8      softmax over k-pixels
        o_h = attn @ v_h
    y = Wout @ o + bout + x_self

Sharding: pure data-parallel over batch; B == 8 == n_cores, each NeuronCore
computes one batch element end-to-end with replicated weights. No collectives.

On-device layout (per core):
    x_self, x_cross : [C=512, N=1024]   (channels on partitions)
    Q, K            : [512, 1024]       q/k channel-major (head h rows h*64..)
    VT_aug          : [N=1024, 8*(64+1)] v transposed, per-head 64 cols + ones
                      column (ones column makes the U matmul also emit the
                      softmax denominator as output row 64)
    scores^T        : [m=1024, n=1024] per head, m on partitions -> softmax
                      denominator computed by PE via the ones column; exp on ACT
    U = [v|1]^T E   : [65, 1024] psum; row 64 = sum_m exp(scores^T[m, n])
    O = U[0:64] / S : normalize via base-0 S hop + reciprocal + gpsimd bcast
    y = WoutT^T O + bout' + x_self,  bout' = bout + Wout@bv (folded on host)

bk is dropped: it shifts every score of a softmax row by the same constant
(softmax invariant). bv is folded into bout' because attention rows sum to 1.

Head pairs (2p, 2p+1) share Q/K row-tiles; their K=64 score matmuls are issued
back-to-back at partition bases 0/64 so the PE runs them concurrently in
disjoint row groups. The attention loop is software-pipelined one pair ahead:
while pair p's U matmuls accumulate (m-tile at a time), pair p+1's scores and
exps stream, keeping both PE and ACT dense.

Numerics knobs (env):
    ATT_MM = f32r | f32 | bf16   dtype of projection/score matmuls
    ATT_VE = bf16 | f32r | f32   dtype of V/E/O/out-proj matmul path
"""

import os
from contextlib import ExitStack

import ml_dtypes
import numpy as np

import concourse.bass as bass
import concourse.tile as tile
from concourse import bacc, mybir

# Problem dims (fixed by the harness problem)
B = 8
C = 512  # QUERY_DIM == CROSS_DIM == INNER
HEADS = 8
HD = 64
N = 1024  # 32*32 pixels
N_CORES = 8
HDP = HD + 1  # per-head cols in VT_aug (64 v-cols + 1 ones col)

F32 = mybir.dt.float32
F32R = mybir.dt.float32r
BF16 = mybir.dt.bfloat16


def _storage(dt_name):
    if dt_name == "bf16":
        return BF16
    if dt_name == "f32r":
        return F32R
    return F32


def _np_storage(dt_name):
    return ml_dtypes.bfloat16 if dt_name == "bf16" else np.float32


def build(mm="bf16", ve="bf16"):
    nc = bacc.Bacc(
        "TRN2", target_bir_lowering=False, debug=False, num_devices=N_CORES
    )
    mdt = _storage(mm)  # x, Wq/Wk/Wv, Q, K storage
    vdt = _storage(ve)  # VT_aug, E, O, WoutT storage

    xs_d = nc.dram_tensor("x_self", [C, N], mdt, kind="ExternalInput").ap()
    xc_d = nc.dram_tensor("x_cross", [C, N], mdt, kind="ExternalInput").ap()
    wq_d = nc.dram_tensor("wqT", [C, C], mdt, kind="ExternalInput").ap()
    wk_d = nc.dram_tensor("wkT", [C, C], mdt, kind="ExternalInput").ap()
    wv_d = nc.dram_tensor("wvT", [C, C], mdt, kind="ExternalInput").ap()
    wo_d = nc.dram_tensor("woutT", [C, C], vdt, kind="ExternalInput").ap()
    bq_d = nc.dram_tensor("bq", [C], F32, kind="ExternalInput").ap()
    bo_d = nc.dram_tensor("bout2", [C], F32, kind="ExternalInput").ap()
    need_resid = mm == "bf16"
    if need_resid:
        rs_d = nc.dram_tensor("resid", [C, N], F32, kind="ExternalInput").ap()
    y_d = nc.dram_tensor("y", [C, N], F32, kind="ExternalOutput").ap()

    MUL = mybir.AluOpType.mult
    ADD = mybir.AluOpType.add
    EXP = mybir.ActivationFunctionType.Exp

    with tile.TileContext(nc) as tc, ExitStack() as ctx:
        persist = ctx.enter_context(tc.tile_pool(name="persist", bufs=1))
        ppool = ctx.enter_context(tc.tile_pool(name="psum", bufs=1, space="PSUM"))
        epool = ctx.enter_context(
            tc.tile_pool(name="epool", bufs=24 if vdt == BF16 else 9)
        )
        npool = ctx.enter_context(tc.tile_pool(name="norm", bufs=2))
        ypool = ctx.enter_context(tc.tile_pool(name="yout", bufs=2))

        def load(name, src, shape, dtype, split=1):
            t = persist.tile(shape, dtype, tag=name, name=name)
            w = shape[1] // split
            for j in range(split):
                nc.sync.dma_start(t[:, j * w : (j + 1) * w],
                                  src[:, j * w : (j + 1) * w])
            return t

        # ---- persistent loads (tiny biases first, then in first-use order)
        bq_s = load("bq", bq_d.rearrange("(a p) -> p a", p=128), [128, 4], F32)
        bo_s = load("bo", bo_d.rearrange("(a p) -> p a", p=128), [128, 4], F32)
        wk_s = [load(f"wk{i}", wk_d[i * 128 : (i + 1) * 128, :], [128, C], mdt)
                for i in range(4)]
        xc_s = [load(f"xc{i}", xc_d[i * 128 : (i + 1) * 128, :], [128, N], mdt)
                for i in range(4)]
        wq_s = [load(f"wq{i}", wq_d[i * 128 : (i + 1) * 128, :], [128, C], mdt)
                for i in range(4)]
        xs_s = [load(f"xs{i}", xs_d[i * 128 : (i + 1) * 128, :], [128, N], mdt)
                for i in range(4)]
        wv_s = [load(f"wv{i}", wv_d[i * 128 : (i + 1) * 128, :], [128, C], mdt)
                for i in range(4)]
        wo_s = [load(f"wo{i}", wo_d[i * 128 : (i + 1) * 128, :], [128, C], vdt)
                for i in range(4)]
        if need_resid:
            rs_s = [load(f"rs{i}", rs_d[i * 128 : (i + 1) * 128, :], [128, N], F32)
                    for i in range(4)]
        elif mm == "f32r":
            # f32r storage holds full fp32 bits; view as fp32 for the residual
            rs_s = None
        else:
            rs_s = xs_s

        # Warm the exp table set early so the first real exp doesn't pay the
        # ~2.7us ACT_TABLE_LOAD on the critical path.
        warm = npool.tile([1, 8], F32, tag="warm", name="warm", bufs=1)
        nc.scalar.activation(warm[:, 0:4], bq_s[0:1, 0:4], EXP)

        # VT_aug tiles: per-head [64 v-cols | ones] blocks
        vt_s = [persist.tile([128, HEADS * HDP], vdt, tag=f"vt{t}", name=f"vt{t}")
                for t in range(8)]
        for t in range(8):
            ones_cols = vt_s[t][:].rearrange("p (h d) -> p h d", d=HDP)[
                :, :, HD : HD + 1
            ]
            nc.gpsimd.memset(ones_cols, 1.0)

        # ---- Q / K projection step (one m-tile) -------------------------
        q_s, k_s = {}, {}
        def qk_proj_step(which, mt):
            w_s, x_in, dst = (
                (wq_s, xs_s, q_s) if which == "q" else (wk_s, xc_s, k_s)
            )
            t = persist.tile([128, N], mdt, tag=f"{which}{mt}",
                             name=f"{which}{mt}")
            for nh in range(2):
                psh = ppool.tile([128, 512], F32, tag="ubank", name="psh",
                                 bufs=4)
                for ct in range(4):
                    nc.tensor.matmul(
                        psh[:, 0:512],
                        lhsT=w_s[ct][:, mt * 128 : (mt + 1) * 128],
                        rhs=x_in[ct][:, nh * 512 : (nh + 1) * 512],
                        start=(ct == 0),
                        stop=(ct == 3),
                    )
                dsth = t[:, nh * 512 : (nh + 1) * 512]
                if which == "q":
                    nc.vector.tensor_scalar_add(dsth, psh[:], bq_s[:, mt : mt + 1])
                else:
                    nc.vector.tensor_copy(out=dsth, in_=psh[:])
            dst[mt] = t

        # ---- V^T projection (x_cross^T @ Wv^T), emitted per m-tile ------
        def vt_proj_step(t):
            ps = ppool.tile([128, 512], F32, tag="ubank", name="vps", bufs=4)
            for ct in range(4):
                nc.tensor.matmul(
                    ps[:, 0:512],
                    lhsT=xc_s[ct][:, t * 128 : (t + 1) * 128],
                    rhs=wv_s[ct][:, :],
                    start=(ct == 0),
                    stop=(ct == 3),
                )
            vsrc = ps[:, 0:512].rearrange("p (h d) -> p h d", d=HD)
            vdst = vt_s[t][:].rearrange("p (h d) -> p h d", d=HDP)[:, :, 0:HD]
            nc.vector.tensor_copy(out=vdst, in_=vsrc)

        # ---- attention: pair-pipelined ----------------------------------
        o_s = [persist.tile([128, N], vdt, tag=f"o{i}", name=f"o{i}")
               for i in range(4)]

        def scores_step(p, t):
            """scores + exp for heads (2p, 2p+1) at m-tile t.
            Each n-half gets ONE psum tile holding both heads (a -> bank 0,
            b -> bank 1); the two K=64 matmuls sit at partition bases 0/64 and
            are issued back-to-back so the PE overlaps them in disjoint row
            groups, and one exp covers both. Returns E tiles [(t,nh)] with
            head a in cols 0:512 and head b in cols 512:1024."""
            es = []
            for nh in range(2):
                ps = ppool.tile([128, N], F32, tag="sc", name="sc", bufs=2)
                for i, base in enumerate((0, 64)):
                    nc.tensor.matmul(
                        ps[:, i * 512 : (i + 1) * 512],
                        lhsT=k_s[p][base : base + 64, t * 128 : (t + 1) * 128],
                        rhs=q_s[p][base : base + 64, nh * 512 : (nh + 1) * 512],
                        start=True,
                        stop=True,
                    )
                e = epool.tile([128, N], vdt, tag="e", name="e")
                nc.scalar.activation(e[:], ps[:], EXP, scale=0.125)
                es.append(e)
            return es

        def u_mm(p, ups, e_pair, i, nh, m):
            """One U accumulation matmul: head i of pair p, n-half nh,
            m-tile m, into the bank tile ups[(i, nh)]."""
            h = 2 * p + i
            nc.tensor.matmul(
                ups[(i, nh)][0:65, 0:512],
                lhsT=vt_s[m][:, h * HDP : (h + 1) * HDP],
                rhs=e_pair[m][nh][:, i * 512 : (i + 1) * 512],
                start=(m == 0),
                stop=(m == 7),
            )

        def normalize_half(p, ups, nh, s0_on_act=False):
            """O halves for heads (2p, 2p+1), n-half nh: U[0:64]/bcast(U[64]).
            Emitted as soon as the two banks' accumulations stop, so the DVE
            chain overlaps the remaining U matmuls of the pair."""
            lo, hi = nh * 512, (nh + 1) * 512
            s0s, r0s, Rs = [], [], []
            for i in range(2):
                s0 = npool.tile([1, 512], F32, tag=f"s0_{i}", name="s0", bufs=1)
                if s0_on_act:
                    nc.scalar.copy(s0[:], ups[(i, nh)][64:65, 0:512])
                else:
                    nc.vector.tensor_copy(out=s0[:], in_=ups[(i, nh)][64:65, 0:512])
                s0s.append(s0)
            for i in range(2):
                r0 = npool.tile([1, 512], F32, tag=f"r0_{i}", name="r0", bufs=1)
                nc.vector.reciprocal_approx_fast(r0[:], s0s[i][:])
                r0s.append(r0)
            for i in range(2):
                R = npool.tile([64, 512], F32, tag=f"R{i}", name="R", bufs=1)
                nc.gpsimd.partition_broadcast(R[:], r0s[i][:])
                Rs.append(R)
            for i in range(2):
                if i == 0:
                    nc.vector.tensor_tensor(
                        o_s[p][0:64, lo:hi], ups[(i, nh)][0:64, 0:512], Rs[i][:],
                        op=MUL,
                    )
                else:
                    stg = npool.tile([64, 512], vdt, tag="stg", name="stg")
                    nc.vector.tensor_tensor(
                        stg[:], ups[(i, nh)][0:64, 0:512], Rs[i][:], op=MUL
                    )
                    nc.vector.tensor_copy(out=o_s[p][64:128, lo:hi], in_=stg[:])

        # Only pair 0's Q/K tiles are produced up front; the remaining six
        # projection m-tiles and the V^T projection are injected into the
        # prologue's exp-paced gaps, so attention starts as soon as the first
        # 3 MB of inputs have landed.
        # mt=0 K/Q emitted per n-half, K-half before Q-half, so the first
        # scores matmuls unblock after only half the projection work
        k0 = persist.tile([128, N], mdt, tag="k0", name="k0")
        q0 = persist.tile([128, N], mdt, tag="q0", name="q0")
        k_s[0], q_s[0] = k0, q0
        for nh in range(2):
            for which, w_s, x_in, t in (("k", wk_s, xc_s, k0),
                                        ("q", wq_s, xs_s, q0)):
                psh = ppool.tile([128, 512], F32, tag="ubank", name="psh",
                                 bufs=4)
                for ct in range(4):
                    nc.tensor.matmul(
                        psh[:, 0:512],
                        lhsT=w_s[ct][:, 0:128],
                        rhs=x_in[ct][:, nh * 512 : (nh + 1) * 512],
                        start=(ct == 0),
                        stop=(ct == 3),
                    )
                dsth = t[:, nh * 512 : (nh + 1) * 512]
                if which == "q":
                    nc.vector.tensor_scalar_add(dsth, psh[:], bq_s[:, 0:1])
                else:
                    nc.vector.tensor_copy(out=dsth, in_=psh[:])
        inject = [("q", 1), ("k", 1)]
        e_prev = []
        for t in range(8):
            e_prev.append(scores_step(0, t))
            vt_proj_step(t)
            if t < len(inject):
                qk_proj_step(*inject[t])

        # During the last pair's loop the scores ring is idle, so Y psums for
        # m-tiles 0/1 open there and accumulate head-pairs 0..2 early.
        y_ps = {}
        y_half = {}
        y_partial = [(mt, nh, dt_) for mt in (0, 1) for nh in range(2)
                     for dt_ in range(3)]

        def y_mm(mt, nh, dt_, start, stop):
            nc.tensor.matmul(
                y_ps[mt][:, nh * 512 : (nh + 1) * 512],
                lhsT=wo_s[dt_][:, mt * 128 : (mt + 1) * 128],
                rhs=o_s[dt_][:, nh * 512 : (nh + 1) * 512],
                start=start,
                stop=stop,
            )

        for p in range(4):
            ups = {}
            e_next = []
            for ti in range(8):
                if p + 1 < 4:
                    e_next.append(scores_step(p + 1, ti))
                if 1 <= ti <= 4:
                    # nh0 banks accumulate over ti 1..4 and stop early, so
                    # their normalize chains overlap the nh1 matmuls
                    for i in range(2):
                        if (i, 0) not in ups:
                            ups[(i, 0)] = ppool.tile([65, 512], F32,
                                                     tag="ubank", name="ub0",
                                                     bufs=4)
                        u_mm(p, ups, e_prev, i, 0, 2 * (ti - 1))
                        u_mm(p, ups, e_prev, i, 0, 2 * ti - 1)
                elif ti >= 5:
                    for i in range(2):
                        if (i, 1) not in ups:
                            ups[(i, 1)] = ppool.tile([65, 512], F32,
                                                     tag="ubank", name="ub1",
                                                     bufs=4)
                        u_mm(p, ups, e_prev, i, 1, 2 * (ti - 5))
                        u_mm(p, ups, e_prev, i, 1, 2 * ti - 9)
                if ti == 4:
                    normalize_half(p, ups, 0, s0_on_act=(p == 3))
                if p < 2 and ti in (4, 5):
                    qk_proj_step("q" if ti == 4 else "k", p + 2)
            for i in range(2):
                u_mm(p, ups, e_prev, i, 1, 6)
                u_mm(p, ups, e_prev, i, 1, 7)
            if p == 3:
                # open Y psums and accumulate head-pairs 0..2 while the
                # normalize chain for the last pair runs on DVE/GpSimd
                for mt in (0, 1):
                    y_ps[mt] = ppool.tile([128, N], F32, tag="sc",
                                          name="yps", bufs=2)
                for mt, nh, dt_ in y_partial:
                    y_mm(mt, nh, dt_, start=(dt_ == 0), stop=False)
                # mt 2/3 accumulate dt 0..2 in half-width psums from the
                # "ubank" slots the nh0 normalize has just freed
                for mt in (2, 3):
                    for nh in range(2):
                        y_half[(mt, nh)] = ppool.tile([128, 512], F32,
                                                      tag="ubank",
                                                      name="yph", bufs=4)
                for dt_ in range(3):
                    for mt in (2, 3):
                        for nh in range(2):
                            nc.tensor.matmul(
                                y_half[(mt, nh)][:, 0:512],
                                lhsT=wo_s[dt_][:, mt * 128 : (mt + 1) * 128],
                                rhs=o_s[dt_][:, nh * 512 : (nh + 1) * 512],
                                start=(dt_ == 0),
                                stop=False,
                            )
            normalize_half(p, ups, 1, s0_on_act=(p == 3))
            e_prev = e_next

        # ---- output projection + bias + residual ------------------------
        def y_finish(mt, ps):
            y = ypool.tile([128, N], F32, tag="y", name="y_t")
            if rs_s is None:
                resid_ap = xs_s[mt][:].bitcast(F32)
            else:
                resid_ap = rs_s[mt][:]
            nc.vector.scalar_tensor_tensor(
                y[:], ps[:], bo_s[:, mt : mt + 1], resid_ap, op0=ADD, op1=ADD
            )
            nc.sync.dma_start(y_d[mt * 128 : (mt + 1) * 128, :], y[:])

        for mt in (0, 1):
            for nh in range(2):
                y_mm(mt, nh, 3, start=False, stop=True)
            y_finish(mt, y_ps[mt])
        for mt in (2, 3):
            for nh in range(2):
                nc.tensor.matmul(
                    y_half[(mt, nh)][:, 0:512],
                    lhsT=wo_s[3][:, mt * 128 : (mt + 1) * 128],
                    rhs=o_s[3][:, nh * 512 : (nh + 1) * 512],
                    start=False,
                    stop=True,
                )
            y = ypool.tile([128, N], F32, tag="y", name="y_t")
            if rs_s is None:
                resid_ap = xs_s[mt][:].bitcast(F32)
            else:
                resid_ap = rs_s[mt][:]
            for nh in range(2):
                nc.vector.scalar_tensor_tensor(
                    y[:, nh * 512 : (nh + 1) * 512],
                    y_half[(mt, nh)][:, 0:512],
                    bo_s[:, mt : mt + 1],
                    resid_ap[:, nh * 512 : (nh + 1) * 512]
                    if rs_s is not None
                    else xs_s[mt][:, nh * 512 : (nh + 1) * 512].bitcast(F32),
                    op0=ADD, op1=ADD,
                )
            nc.sync.dma_start(y_d[mt * 128 : (mt + 1) * 128, :], y[:])

    nc.compile()
    return nc


_CACHE = {}


def get_nc(mm=None, ve=None):
    mm = mm or os.environ.get("ATT_MM", "bf16")
    ve = ve or os.environ.get("ATT_VE", "bf16")
    key = (mm, ve)
    if key not in _CACHE:
        _CACHE[key] = build(*key)
    return _CACHE[key], key


def make_in_maps(self_feature, cross_feature, Wq, bq, Wk, bk, Wv, bv, Wout, bout,
                 mm, ve):
    f32 = np.float32
    np_m = _np_storage(mm)
    np_v = _np_storage(ve)
    sf = np.asarray(self_feature, f32).reshape(B, C, N)
    cf = np.asarray(cross_feature, f32).reshape(B, C, N)
    Wq = np.asarray(Wq, f32)
    Wk = np.asarray(Wk, f32)
    Wv = np.asarray(Wv, f32)
    Wout = np.asarray(Wout, f32)
    wqT = np.ascontiguousarray(Wq.T).astype(np_m)
    wkT = np.ascontiguousarray(Wk.T).astype(np_m)
    wvT = np.ascontiguousarray(Wv.T).astype(np_m)
    woT = np.ascontiguousarray(Wout.T).astype(np_v)
    bq = np.asarray(bq, f32)
    bout2 = (np.asarray(bout, f32) + Wout @ np.asarray(bv, f32)).astype(f32)
    # bk is intentionally unused: adding k-bias shifts all scores in a softmax
    # row by the same amount, which softmax cancels exactly.
    del bk
    in_maps = []
    for b in range(B):
        m = {
            "x_self": np.ascontiguousarray(sf[b]).astype(np_m),
            "x_cross": np.ascontiguousarray(cf[b]).astype(np_m),
            "wqT": wqT, "wkT": wkT, "wvT": wvT, "woutT": woT,
            "bq": bq, "bout2": bout2,
        }
        if mm == "bf16":
            m["resid"] = np.ascontiguousarray(sf[b])
        in_maps.append(m)
    return in_maps


def kernel(self_feature, cross_feature, Wq, bq, Wk, bk, Wv, bv, Wout, bout):
    from concourse.bass_utils import run_bass_kernel_spmd

    nc, (mm, ve) = get_nc()
    in_maps = make_in_maps(self_feature, cross_feature, Wq, bq, Wk, bk, Wv, bv,
                           Wout, bout, mm, ve)
    res = run_bass_kernel_spmd(nc, in_maps, core_ids=list(range(N_CORES)))
    y = np.stack([res.results[b]["y"].reshape(C, 32, 32) for b in range(B)])
    return np.ascontiguousarray(y.astype(np.float32))
